# revision 2
# baseline (speedup 1.0000x reference)
"""Trainium2 Bass kernel for CrossTokenMLPAggregator (top-k masked attention aggregation).

Computes, for full inputs
    mlp_hidden   [B=2, T=2048, H=1024] f32
    attn_weights [B=2, Hh=16, T=2048, T=2048] f32
the reference:
    W = attn_weights.mean(axis=1)              # [B, T, T]
    keep top-8 per query row, renormalize kept mass to sum 1
    out = einsum('bts,bsh->bth', W_sparse, mlp_hidden)

Sharding: 8 cores, each owns 512 query rows (core c -> batch c//4,
query rows (c%4)*512 ...). Each core streams its [16, 512, 2048] slice of
attn_weights (the 512 MiB input dominates; split is exact, no duplication),
accumulates the head sum sequentially on DVE (bit-exact with XLA's
sequential-h mean order, so top-8 selection matches the reference), finds
the top-8 via the DVE max8 instruction, masks with (W >= v8)*W in one
scalar_tensor_tensor op, transposes the masked rows on the TensorEngine
and contracts in bf16 (PSUM fp32 accumulate; well within the 2e-2 rel-err
budget) with the bf16 mlp_hidden slice resident in SBUF. Renormalization
(1/sum of kept fp32 mass) is folded into the PSUM->SBUF eviction on the
ScalarEngine. bf16 matmuls run at 4x the fp32 rate, keeping the
TensorEngine (and power throttling) far off the critical path; the DMA
stream is the bottleneck at ~395 GB/s/core sustained. The last tile's
head-sum is split into s-halves so its top-8 scan of the first half hides
under the second half's streaming, shortening the post-stream tail.
"""

import numpy as np

B, T, H, Hh, K = 2, 2048, 1024, 16, 8
NCORES = 8
QPC = (B * T) // NCORES          # 512 query rows per core
P = 128                          # partitions
TQ_TILES = QPC // P              # 4 tiles of 128 query rows
S_CHUNKS = T // P                # 16 contraction chunks
EPS_SUM = np.float32(1e-8) * np.float32(16.0)  # EPS in head-sum domain

_compiled = {}


def _build_nc():
    import concourse.bass as bass
    import concourse.bacc as bacc
    import concourse.mybir as mybir
    import concourse.tile as tile
    from concourse import masks

    f32 = mybir.dt.float32
    bf16 = mybir.dt.bfloat16
    nc = bacc.Bacc(
        "TRN2",
        target_bir_lowering=False,
        debug=False,
        enable_asserts=False,
        num_devices=NCORES,
    )
    attn = nc.dram_tensor("attn", [Hh, QPC, T], f32, kind="ExternalInput").ap()
    mlp = nc.dram_tensor("mlp", [T, H], f32, kind="ExternalInput").ap()
    out = nc.dram_tensor("out", [QPC, H], f32, kind="ExternalOutput").ap()

    with tile.TileContext(nc) as tc:
        with (
            tc.tile_pool(name="persist", bufs=1) as persist,
            tc.tile_pool(name="heads", bufs=12) as heads,
            tc.tile_pool(name="acc", bufs=2) as accp,
            tc.tile_pool(name="wm", bufs=2) as wmp,
            tc.tile_pool(name="wmt", bufs=2) as wmtp,
            tc.tile_pool(name="small", bufs=2) as small,
            tc.tile_pool(name="outsb", bufs=2) as outsbp,
            tc.tile_pool(name="tp_psum", bufs=4, space="PSUM") as tp_psum,
            tc.tile_pool(name="mm_psum", bufs=2, space="PSUM") as mm_psum,
        ):
            # mlp_hidden slice resident in SBUF as bf16: [128, 16 chunks, 1024].
            # SWDGE cast-DMA converts fp32->bf16 in flight; per-chunk DMAs keep
            # each source read contiguous (512 KiB).
            mlp_sb = persist.tile([P, S_CHUNKS, H], bf16)
            for c in range(S_CHUNKS):
                nc.gpsimd.dma_start(
                    out=mlp_sb[:, c, :], in_=mlp[c * P : (c + 1) * P, :]
                )
            ident = persist.tile([P, P], f32)
            masks.make_identity(nc, ident[:])

            def transpose_chunks(wm, wmt, c0, c1):
                # fp32 masked rows -> bf16 transposed chunks (cast on the
                # ScalarEngine during PSUM eviction)
                for g in range(c0 // 4, c1 // 4):
                    pt = tp_psum.tile([P, 4 * P], f32, tag="pt")
                    for j in range(4):
                        c = 4 * g + j
                        nc.tensor.transpose(
                            pt[:, j * P : (j + 1) * P],
                            wm[:, c * P : (c + 1) * P],
                            ident[:],
                        )
                    nc.scalar.copy(wmt[:, 4 * g : 4 * g + 4, :], pt[:])

            def matmul_evict_store(wmt, rcp, q):
                acc_ps = mm_psum.tile([P, H], f32, tag="acc_ps")
                osb = outsbp.tile([P, H], f32, tag="osb")
                for nh in range(H // 512):
                    nsl = slice(nh * 512, (nh + 1) * 512)
                    for c in range(S_CHUNKS):
                        nc.tensor.matmul(
                            acc_ps[:, nsl],
                            lhsT=wmt[:, c, :],
                            rhs=mlp_sb[:, c, nsl],
                            start=(c == 0),
                            stop=(c == S_CHUNKS - 1),
                        )
                    # renormalize + evict this half on ScalarE
                    nc.scalar.activation(
                        out=osb[:, nsl],
                        in_=acc_ps[:, nsl],
                        func=mybir.ActivationFunctionType.Copy,
                        scale=rcp[:, :],
                    )
                    nc.sync.dma_start(out=out[q, nsl], in_=osb[:, nsl])

            def mask_renorm(acc, wm, v8):
                # wm = (acc >= v8) * acc ; ssum = sum(wm) ; rcp = 1/max(ssum,eps)
                ssum = small.tile([P, 1], f32, tag="ssum")
                nc.vector.scalar_tensor_tensor(
                    out=wm,
                    in0=acc,
                    scalar=v8,
                    in1=acc,
                    op0=mybir.AluOpType.is_ge,
                    op1=mybir.AluOpType.mult,
                    accum_out=ssum,
                )
                nc.vector.tensor_scalar_max(ssum, ssum, float(EPS_SUM))
                rcp = small.tile([P, 1], f32, tag="rcp")
                nc.vector.reciprocal(rcp, ssum)
                return rcp

            for t in range(TQ_TILES - 1):
                q = slice(t * P, (t + 1) * P)
                # ---- head-sum accumulation (sequential in h; order matters
                # for bit-exact top-8 selection vs the reference mean) ----
                acc = accp.tile([P, T], f32)
                nc.sync.dma_start(out=acc, in_=attn[0, q, :])
                for h in range(1, Hh):
                    ht = heads.tile([P, T], f32, tag="ht")
                    nc.sync.dma_start(out=ht, in_=attn[h, q, :])
                    nc.vector.tensor_add(out=acc, in0=acc, in1=ht)

                # ---- top-8 values per row ----
                mx = small.tile([P, K], f32, tag="mx")
                nc.vector.max(out=mx, in_=acc)

                wm = wmp.tile([P, T], f32, tag="wm")
                rcp = mask_renorm(acc, wm, mx[:, K - 1 : K])

                wmt = wmtp.tile([P, S_CHUNKS, P], bf16, tag="wmt")
                transpose_chunks(wm, wmt, 0, S_CHUNKS)
                matmul_evict_store(wmt, rcp, q)

            # ---- last tile: stream s-halves so the first half's top-8 scan
            # hides under the second half's streaming; merge the two local
            # top-8 lists for the global threshold, then mask/contract. ----
            t = TQ_TILES - 1
            q = slice(t * P, (t + 1) * P)
            HF = T // 2
            acc = accp.tile([P, T], f32)
            nc.sync.dma_start(out=acc[:, :HF], in_=attn[0, q, :HF])
            for h in range(1, Hh):
                ht = heads.tile([P, HF], f32, tag="ht")
                nc.sync.dma_start(out=ht, in_=attn[h, q, :HF])
                nc.vector.tensor_add(out=acc[:, :HF], in0=acc[:, :HF], in1=ht)
            mx1 = small.tile([P, K], f32, tag="mx")
            nc.vector.max(out=mx1, in_=acc[:, :HF])
            nc.sync.dma_start(out=acc[:, HF:], in_=attn[0, q, HF:])
            for h in range(1, Hh):
                ht = heads.tile([P, HF], f32, tag="ht")
                nc.sync.dma_start(out=ht, in_=attn[h, q, HF:])
                nc.vector.tensor_add(out=acc[:, HF:], in0=acc[:, HF:], in1=ht)
            mx2 = small.tile([P, K], f32, tag="mx2")
            nc.vector.max(out=mx2, in_=acc[:, HF:])
            mg = small.tile([P, 2 * K], f32, tag="mg")
            nc.vector.tensor_copy(mg[:, :K], mx1)
            nc.vector.tensor_copy(mg[:, K:], mx2)
            g8 = small.tile([P, K], f32, tag="g8")
            nc.vector.max(out=g8, in_=mg)

            wm = wmp.tile([P, T], f32, tag="wm")
            rcp = mask_renorm(acc, wm, g8[:, K - 1 : K])
            wmt = wmtp.tile([P, S_CHUNKS, P], bf16, tag="wmt")
            transpose_chunks(wm, wmt, 0, S_CHUNKS)
            matmul_evict_store(wmt, rcp, q)

    nc.compile()
    return nc


def _get_nc():
    if "nc" not in _compiled:
        _compiled["nc"] = _build_nc()
    return _compiled["nc"]


def kernel(mlp_hidden: np.ndarray, attn_weights: np.ndarray) -> np.ndarray:
    from concourse.bass_utils import run_bass_kernel_spmd

    mlp_hidden = np.ascontiguousarray(mlp_hidden, dtype=np.float32)
    attn_weights = np.ascontiguousarray(attn_weights, dtype=np.float32)
    assert mlp_hidden.shape == (B, T, H)
    assert attn_weights.shape == (B, Hh, T, T)

    nc = _get_nc()
    in_maps = []
    for c in range(NCORES):
        b = c // (NCORES // B)
        q0 = (c % (NCORES // B)) * QPC
        in_maps.append(
            {
                "attn": np.ascontiguousarray(attn_weights[b, :, q0 : q0 + QPC, :]),
                "mlp": mlp_hidden[b],
            }
        )
    res = run_bass_kernel_spmd(nc, in_maps, list(range(NCORES)))
    out = np.empty((B, T, H), dtype=np.float32)
    for c in range(NCORES):
        b = c // (NCORES // B)
        q0 = (c % (NCORES // B)) * QPC
        out[b, q0 : q0 + QPC] = res.results[c]["out"]
    return out


# revision 5
# speedup vs baseline: 1.1075x; 1.1075x over previous
"""Trainium2 Bass kernel for CrossTokenMLPAggregator (top-k masked attention aggregation).

Computes, for full inputs
    mlp_hidden   [B=2, T=2048, H=1024] f32
    attn_weights [B=2, Hh=16, T=2048, T=2048] f32
the reference:
    W = attn_weights.mean(axis=1)              # [B, T, T]
    keep top-8 per query row, renormalize kept mass to sum 1
    out = einsum('bts,bsh->bth', W_sparse, mlp_hidden)

Sharding: 8 cores, each owns 512 query rows (core c -> batch c//4,
query rows (c%4)*512 ...). Each core streams its [16, 512, 2048] slice of
attn_weights (the 512 MiB input dominates; split is exact, no duplication),
accumulates the head sum sequentially on DVE (bit-exact with XLA's
sequential-h mean order, so top-8 selection matches the reference), finds
the top-8 via the DVE max8 instruction, masks with (W >= v8)*W in one
scalar_tensor_tensor op, transposes the masked rows on the TensorEngine
and contracts in bf16 (PSUM fp32 accumulate; well within the 2e-2 rel-err
budget) with the bf16 mlp_hidden slice resident in SBUF. Renormalization
(1/sum of kept fp32 mass) is folded into the PSUM->SBUF eviction on the
ScalarEngine. bf16 matmuls run at 4x the fp32 rate, keeping the
TensorEngine (and power throttling) far off the critical path; the DMA
stream is the bottleneck at ~395 GB/s/core sustained. The last tile's
head-sum is split into s-halves so its top-8 scan of the first half hides
under the second half's streaming, shortening the post-stream tail.
"""

import numpy as np

B, T, H, Hh, K = 2, 2048, 1024, 16, 8
NCORES = 8
QPC = (B * T) // NCORES          # 512 query rows per core
P = 128                          # partitions
TQ_TILES = QPC // P              # 4 tiles of 128 query rows
S_CHUNKS = T // P                # 16 contraction chunks
EPS_SUM = np.float32(1e-8) * np.float32(16.0)  # EPS in head-sum domain

_compiled = {}


def _build_nc():
    import concourse.bass as bass
    import concourse.bacc as bacc
    import concourse.mybir as mybir
    import concourse.tile as tile
    from concourse import masks

    f32 = mybir.dt.float32
    bf16 = mybir.dt.bfloat16
    nc = bacc.Bacc(
        "TRN2",
        target_bir_lowering=False,
        debug=False,
        enable_asserts=False,
        num_devices=NCORES,
    )
    attn = nc.dram_tensor("attn", [Hh, QPC, T], f32, kind="ExternalInput").ap()
    mlp = nc.dram_tensor("mlp", [T, H], f32, kind="ExternalInput").ap()
    out = nc.dram_tensor("out", [QPC, H], f32, kind="ExternalOutput").ap()

    with tile.TileContext(nc) as tc:
        with (
            tc.tile_pool(name="persist", bufs=1) as persist,
            tc.tile_pool(name="heads", bufs=15) as heads,
            tc.tile_pool(name="acc", bufs=2) as accp,
            tc.tile_pool(name="wm", bufs=1) as wmp,
            tc.tile_pool(name="wmt", bufs=1) as wmtp,
            tc.tile_pool(name="small", bufs=2) as small,
            tc.tile_pool(name="outsb", bufs=2) as outsbp,
            tc.tile_pool(name="tp_psum", bufs=4, space="PSUM") as tp_psum,
            tc.tile_pool(name="mm_psum", bufs=2, space="PSUM") as mm_psum,
        ):
            # mlp_hidden slice resident in SBUF as bf16: [128, 16 chunks, 1024].
            # SWDGE cast-DMA converts fp32->bf16 in flight; per-chunk DMAs keep
            # each source read contiguous (512 KiB). Issued AFTER tile 0's head
            # DMAs (below) so the 8 MiB load doesn't delay the pipeline start.
            mlp_sb = persist.tile([P, S_CHUNKS, H], bf16)
            ident = persist.tile([P, P], f32)
            masks.make_identity(nc, ident[:])

            def transpose_chunks(wm, wmt, c0, c1):
                # fp32 masked rows -> bf16 transposed chunks (cast on the
                # ScalarEngine during PSUM eviction)
                for g in range(c0 // 4, c1 // 4):
                    pt = tp_psum.tile([P, 4 * P], f32, tag="pt")
                    for j in range(4):
                        c = 4 * g + j
                        nc.tensor.transpose(
                            pt[:, j * P : (j + 1) * P],
                            wm[:, c * P : (c + 1) * P],
                            ident[:],
                        )
                    nc.scalar.copy(wmt[:, 4 * g : 4 * g + 4, :], pt[:])

            def matmul_evict_store(wmt, rcp, q):
                acc_ps = mm_psum.tile([P, H], f32, tag="acc_ps")
                osb = outsbp.tile([P, H], f32, tag="osb")
                for nh in range(H // 512):
                    nsl = slice(nh * 512, (nh + 1) * 512)
                    for c in range(S_CHUNKS):
                        nc.tensor.matmul(
                            acc_ps[:, nsl],
                            lhsT=wmt[:, c, :],
                            rhs=mlp_sb[:, c, nsl],
                            start=(c == 0),
                            stop=(c == S_CHUNKS - 1),
                        )
                    # renormalize + evict this half on ScalarE
                    nc.scalar.activation(
                        out=osb[:, nsl],
                        in_=acc_ps[:, nsl],
                        func=mybir.ActivationFunctionType.Copy,
                        scale=rcp[:, :],
                    )
                    nc.sync.dma_start(out=out[q, nsl], in_=osb[:, nsl])

            def mask_renorm(acc, wm, v8):
                # wm = (acc >= v8) * acc ; ssum = sum(wm) ; rcp = 1/max(ssum,eps)
                ssum = small.tile([P, 1], f32, tag="ssum")
                nc.vector.scalar_tensor_tensor(
                    out=wm,
                    in0=acc,
                    scalar=v8,
                    in1=acc,
                    op0=mybir.AluOpType.is_ge,
                    op1=mybir.AluOpType.mult,
                    accum_out=ssum,
                )
                nc.vector.tensor_scalar_max(ssum, ssum, float(EPS_SUM))
                rcp = small.tile([P, 1], f32, tag="rcp")
                nc.vector.reciprocal(rcp, ssum)
                return rcp

            for t in range(TQ_TILES - 1):
                q = slice(t * P, (t + 1) * P)
                # ---- head-sum accumulation (sequential in h; order matters
                # for bit-exact top-8 selection vs the reference mean) ----
                acc = accp.tile([P, T], f32)
                nc.sync.dma_start(out=acc, in_=attn[0, q, :])
                for h in range(1, Hh):
                    ht = heads.tile([P, T], f32, tag="ht")
                    nc.sync.dma_start(out=ht, in_=attn[h, q, :])
                    nc.vector.tensor_add(out=acc, in0=acc, in1=ht)

                if t == 0:
                    # mlp load goes out behind tile 0's heads; it streams
                    # while the DVE chews through the first add chain and is
                    # resident before tile 0's matmuls need it.
                    for c in range(S_CHUNKS):
                        nc.gpsimd.dma_start(
                            out=mlp_sb[:, c, :], in_=mlp[c * P : (c + 1) * P, :]
                        )

                # ---- top-8 values per row ----
                mx = small.tile([P, K], f32, tag="mx")
                nc.vector.max(out=mx, in_=acc)

                wm = wmp.tile([P, T], f32, tag="wm")
                rcp = mask_renorm(acc, wm, mx[:, K - 1 : K])

                wmt = wmtp.tile([P, S_CHUNKS, P], bf16, tag="wmt")
                transpose_chunks(wm, wmt, 0, S_CHUNKS)
                matmul_evict_store(wmt, rcp, q)

            # ---- last tile: stream s-halves so the first half's top-8 scan
            # hides under the second half's streaming; merge the two local
            # top-8 lists for the global threshold, then mask/contract. ----
            t = TQ_TILES - 1
            q = slice(t * P, (t + 1) * P)
            HF = T // 2
            acc = accp.tile([P, T], f32)
            nc.sync.dma_start(out=acc[:, :HF], in_=attn[0, q, :HF])
            for h in range(1, Hh):
                ht = heads.tile([P, HF], f32, tag="ht")
                nc.sync.dma_start(out=ht, in_=attn[h, q, :HF])
                nc.vector.tensor_add(out=acc[:, :HF], in0=acc[:, :HF], in1=ht)
            mx1 = small.tile([P, K], f32, tag="mx")
            nc.vector.max(out=mx1, in_=acc[:, :HF])
            nc.sync.dma_start(out=acc[:, HF:], in_=attn[0, q, HF:])
            for h in range(1, Hh):
                ht = heads.tile([P, HF], f32, tag="ht")
                nc.sync.dma_start(out=ht, in_=attn[h, q, HF:])
                nc.vector.tensor_add(out=acc[:, HF:], in0=acc[:, HF:], in1=ht)
            mx2 = small.tile([P, K], f32, tag="mx2")
            nc.vector.max(out=mx2, in_=acc[:, HF:])
            mg = small.tile([P, 2 * K], f32, tag="mg")
            nc.vector.tensor_copy(mg[:, :K], mx1)
            nc.vector.tensor_copy(mg[:, K:], mx2)
            g8 = small.tile([P, K], f32, tag="g8")
            nc.vector.max(out=g8, in_=mg)

            wm = wmp.tile([P, T], f32, tag="wm")
            rcp = mask_renorm(acc, wm, g8[:, K - 1 : K])
            wmt = wmtp.tile([P, S_CHUNKS, P], bf16, tag="wmt")
            transpose_chunks(wm, wmt, 0, S_CHUNKS)
            matmul_evict_store(wmt, rcp, q)

    nc.compile()
    return nc


def _get_nc():
    if "nc" not in _compiled:
        _compiled["nc"] = _build_nc()
    return _compiled["nc"]


def kernel(mlp_hidden: np.ndarray, attn_weights: np.ndarray) -> np.ndarray:
    from concourse.bass_utils import run_bass_kernel_spmd

    mlp_hidden = np.ascontiguousarray(mlp_hidden, dtype=np.float32)
    attn_weights = np.ascontiguousarray(attn_weights, dtype=np.float32)
    assert mlp_hidden.shape == (B, T, H)
    assert attn_weights.shape == (B, Hh, T, T)

    nc = _get_nc()
    in_maps = []
    for c in range(NCORES):
        b = c // (NCORES // B)
        q0 = (c % (NCORES // B)) * QPC
        in_maps.append(
            {
                "attn": np.ascontiguousarray(attn_weights[b, :, q0 : q0 + QPC, :]),
                "mlp": mlp_hidden[b],
            }
        )
    res = run_bass_kernel_spmd(nc, in_maps, list(range(NCORES)))
    out = np.empty((B, T, H), dtype=np.float32)
    for c in range(NCORES):
        b = c // (NCORES // B)
        q0 = (c % (NCORES // B)) * QPC
        out[b, q0 : q0 + QPC] = res.results[c]["out"]
    return out


# revision 7
# speedup vs baseline: 1.1089x; 1.0012x over previous
"""Trainium2 Bass kernel for CrossTokenMLPAggregator (top-k masked attention aggregation).

Computes, for full inputs
    mlp_hidden   [B=2, T=2048, H=1024] f32
    attn_weights [B=2, Hh=16, T=2048, T=2048] f32
the reference:
    W = attn_weights.mean(axis=1)              # [B, T, T]
    keep top-8 per query row, renormalize kept mass to sum 1
    out = einsum('bts,bsh->bth', W_sparse, mlp_hidden)

Sharding: 8 cores, each owns 512 query rows (core c -> batch c//4,
query rows (c%4)*512 ...). Each core streams its [16, 512, 2048] slice of
attn_weights (the 512 MiB input dominates; the split is exact), sums the
heads sequentially on DVE (bit-exact with the reference's mean order so
the top-8 selection matches), finds the top-8 with the DVE max8
instruction, masks with (W >= v8)*W in one scalar_tensor_tensor op,
transposes the masked rows on the TensorEngine and contracts in bf16
(fp32 PSUM accumulate; ~0.2% rounding, far inside the tolerance) against
the bf16 mlp_hidden slice resident in SBUF. Renormalization (1/kept fp32
mass) rides the PSUM->SBUF eviction on the ScalarEngine.

The schedule is fully DMA-bound (~420 GB/s/core sustained): per-tile
epilogues (max8/mask/transpose/matmul) are software-pipelined one tile
late so the DVE add chain — whose buffer releases pace the attn stream —
never pauses at tile boundaries; the last tile streams in s-halves so
only its own epilogue remains exposed after the final attn bytes land.
"""

import numpy as np

B, T, H, Hh, K = 2, 2048, 1024, 16, 8
NCORES = 8
QPC = (B * T) // NCORES          # 512 query rows per core
P = 128                          # partitions
TQ_TILES = QPC // P              # 4 tiles of 128 query rows
S_CHUNKS = T // P                # 16 contraction chunks
EPS_SUM = np.float32(1e-8) * np.float32(16.0)  # EPS in head-sum domain

_compiled = {}


def _build_nc():
    import concourse.bass as bass
    import concourse.bacc as bacc
    import concourse.mybir as mybir
    import concourse.tile as tile
    from concourse import masks

    f32 = mybir.dt.float32
    bf16 = mybir.dt.bfloat16
    nc = bacc.Bacc(
        "TRN2",
        target_bir_lowering=False,
        debug=False,
        enable_asserts=False,
        num_devices=NCORES,
    )
    attn = nc.dram_tensor("attn", [Hh, QPC, T], f32, kind="ExternalInput").ap()
    mlp = nc.dram_tensor("mlp", [T, H], f32, kind="ExternalInput").ap()
    out = nc.dram_tensor("out", [QPC, H], f32, kind="ExternalOutput").ap()

    with tile.TileContext(nc) as tc:
        with (
            tc.tile_pool(name="persist", bufs=1) as persist,
            tc.tile_pool(name="heads", bufs=14) as heads,
            tc.tile_pool(name="acc", bufs=3) as accp,
            tc.tile_pool(name="stage", bufs=2) as stagep,
            tc.tile_pool(name="wm", bufs=1) as wmp,
            tc.tile_pool(name="wmt", bufs=1) as wmtp,
            tc.tile_pool(name="small", bufs=2) as small,
            tc.tile_pool(name="outsb", bufs=2) as outsbp,
            tc.tile_pool(name="tp_psum", bufs=4, space="PSUM") as tp_psum,
            tc.tile_pool(name="mm_psum", bufs=2, space="PSUM") as mm_psum,
        ):
            mlp_sb = persist.tile([P, S_CHUNKS, H], bf16)
            ident = persist.tile([P, P], f32)
            masks.make_identity(nc, ident[:])

            def load_mlp():
                # fp32 chunks via HWDGE (SWDGE cast-DMA packets drag the whole
                # SDMA mix down), cast to bf16 on the otherwise-idle ScalarE
                for c in range(S_CHUNKS):
                    st = stagep.tile([P, H], f32, tag="st")
                    nc.sync.dma_start(out=st, in_=mlp[c * P : (c + 1) * P, :])
                    nc.scalar.copy(mlp_sb[:, c, :], st)

            def epilogue(acc, mx, q):
                # mask with the row's 8th-largest, renormalize kept mass,
                # transpose masked rows, contract with mlp, store.
                wm = wmp.tile([P, T], f32, tag="wm")
                ssum = small.tile([P, 1], f32, tag="ssum")
                nc.vector.scalar_tensor_tensor(
                    out=wm,
                    in0=acc,
                    scalar=mx[:, K - 1 : K],
                    in1=acc,
                    op0=mybir.AluOpType.is_ge,
                    op1=mybir.AluOpType.mult,
                    accum_out=ssum,
                )
                nc.vector.tensor_scalar_max(ssum, ssum, float(EPS_SUM))
                rcp = small.tile([P, 1], f32, tag="rcp")
                nc.vector.reciprocal(rcp, ssum)

                wmt = wmtp.tile([P, S_CHUNKS, P], bf16, tag="wmt")
                for g in range(S_CHUNKS // 4):
                    pt = tp_psum.tile([P, 4 * P], f32, tag="pt")
                    for j in range(4):
                        c = 4 * g + j
                        nc.tensor.transpose(
                            pt[:, j * P : (j + 1) * P],
                            wm[:, c * P : (c + 1) * P],
                            ident[:],
                        )
                    nc.scalar.copy(wmt[:, 4 * g : 4 * g + 4, :], pt[:])

                acc_ps = mm_psum.tile([P, H], f32, tag="acc_ps")
                osb = outsbp.tile([P, H], f32, tag="osb")
                for c in range(S_CHUNKS):
                    for nh in range(H // 512):
                        nsl = slice(nh * 512, (nh + 1) * 512)
                        nc.tensor.matmul(
                            acc_ps[:, nsl],
                            lhsT=wmt[:, c, :],
                            rhs=mlp_sb[:, c, nsl],
                            start=(c == 0),
                            stop=(c == S_CHUNKS - 1),
                        )
                for nh in range(H // 512):
                    nsl = slice(nh * 512, (nh + 1) * 512)
                    nc.scalar.activation(
                        out=osb[:, nsl],
                        in_=acc_ps[:, nsl],
                        func=mybir.ActivationFunctionType.Copy,
                        scale=rcp[:, :],
                    )
                    nc.scalar.dma_start(out=out[q, nsl], in_=osb[:, nsl])

            pending = []  # deferred (acc, mx, q) epilogues

            def run_pending():
                if pending:
                    epilogue(*pending.pop())

            def accumulate(acc, q, sl, first_dma):
                if first_dma:
                    nc.sync.dma_start(out=acc[:, sl], in_=attn[0, q, sl])
                for h in range(1, Hh):
                    ht = heads.tile([P, sl.stop - sl.start], f32, tag="ht")
                    nc.sync.dma_start(out=ht, in_=attn[h, q, sl])
                    nc.vector.tensor_add(
                        out=acc[:, sl], in0=acc[:, sl], in1=ht
                    )

            for t in range(TQ_TILES - 1):
                q = slice(t * P, (t + 1) * P)
                acc = accp.tile([P, T], f32, tag="acc")
                accumulate(acc, q, slice(0, T), True)
                if t == 0:
                    load_mlp()
                mx = small.tile([P, K], f32, tag="mx")
                nc.vector.max(out=mx, in_=acc)
                run_pending()
                pending.append((acc, mx, q))

            # last tile: stream s-halves; the first half's top-8 scan and the
            # second-to-last tile's epilogue hide under the second half's
            # streaming, so only this tile's epilogue trails the stream
            t = TQ_TILES - 1
            q = slice(t * P, (t + 1) * P)
            HF = T // 2
            acc = accp.tile([P, T], f32, tag="acc")
            accumulate(acc, q, slice(0, HF), True)
            mx1 = small.tile([P, K], f32, tag="mx")
            nc.vector.max(out=mx1, in_=acc[:, :HF])
            run_pending()
            accumulate(acc, q, slice(HF, T), True)
            mx2 = small.tile([P, K], f32, tag="mx2")
            nc.vector.max(out=mx2, in_=acc[:, HF:])
            mg = small.tile([P, 2 * K], f32, tag="mg")
            nc.vector.tensor_copy(mg[:, :K], mx1)
            nc.vector.tensor_copy(mg[:, K:], mx2)
            g8 = small.tile([P, K], f32, tag="g8")
            nc.vector.max(out=g8, in_=mg)
            epilogue(acc, g8, q)

    nc.compile()
    return nc


def _get_nc():
    if "nc" not in _compiled:
        _compiled["nc"] = _build_nc()
    return _compiled["nc"]


def kernel(mlp_hidden: np.ndarray, attn_weights: np.ndarray) -> np.ndarray:
    from concourse.bass_utils import run_bass_kernel_spmd

    mlp_hidden = np.ascontiguousarray(mlp_hidden, dtype=np.float32)
    attn_weights = np.ascontiguousarray(attn_weights, dtype=np.float32)
    assert mlp_hidden.shape == (B, T, H)
    assert attn_weights.shape == (B, Hh, T, T)

    nc = _get_nc()
    in_maps = []
    for c in range(NCORES):
        b = c // (NCORES // B)
        q0 = (c % (NCORES // B)) * QPC
        in_maps.append(
            {
                "attn": np.ascontiguousarray(attn_weights[b, :, q0 : q0 + QPC, :]),
                "mlp": mlp_hidden[b],
            }
        )
    res = run_bass_kernel_spmd(nc, in_maps, list(range(NCORES)))
    out = np.empty((B, T, H), dtype=np.float32)
    for c in range(NCORES):
        b = c // (NCORES // B)
        q0 = (c % (NCORES // B)) * QPC
        out[b, q0 : q0 + QPC] = res.results[c]["out"]
    return out


# revision 9
# speedup vs baseline: 1.1727x; 1.0576x over previous
"""Trainium2 Bass kernel for CrossTokenMLPAggregator (top-k masked attention aggregation).

Computes, for full inputs
    mlp_hidden   [B=2, T=2048, H=1024] f32
    attn_weights [B=2, Hh=16, T=2048, T=2048] f32
the reference:
    W = attn_weights.mean(axis=1)              # [B, T, T]
    keep top-8 per query row, renormalize kept mass to sum 1
    out = einsum('bts,bsh->bth', W_sparse, mlp_hidden)

Sharding: 8 cores, each owns 512 query rows (core c -> batch c//4,
query rows (c%4)*512 ...). Each core streams its [16, 512, 2048] slice of
attn_weights (the 512 MiB input dominates; the split is exact), sums the
heads sequentially on DVE (bit-exact with the reference's mean order so
the top-8 selection matches), finds the top-8 with the DVE max8
instruction, masks with (W >= v8)*W in one scalar_tensor_tensor op,
transposes the masked rows on the TensorEngine and contracts in bf16
(fp32 PSUM accumulate; ~0.2% rounding, far inside the tolerance) against
the bf16 mlp_hidden slice resident in SBUF. Renormalization (1/kept fp32
mass) rides the PSUM->SBUF eviction on the ScalarEngine.

Schedule notes (the kernel is DMA-bound at ~420 GB/s/core sustained):
- Per-tile epilogues (mask/transpose/matmul/store) are software-pipelined
  one tile late so the DVE add chain — whose head-buffer releases pace the
  attn stream — never pauses at tile boundaries.
- The mlp load is staged fp32 via the Sync HWDGE queue and cast to bf16 on
  the ScalarEngine, in groups of 4 chunks interleaved into the first two
  tiles' add chains: a group of <= stage-bufs triggers never waits on the
  ScalarE casts, so the head-DMA stream behind it in the queue is never
  head-of-line blocked (SWDGE cast-DMA and a monolithic staged load both
  measurably stall the stream).
- Output DMAs issue from the ScalarEngine's HWDGE queue right after the
  eviction they depend on, keeping the Sync queue free for head DMAs.
- The last tile streams in s-halves (its first-half top-8 scan and the
  previous tile's epilogue hide under the second half's streaming), and
  its mask/transpose run half-by-half so only ~the matmul block trails
  the final attn bytes.
"""

import numpy as np

B, T, H, Hh, K = 2, 2048, 1024, 16, 8
NCORES = 8
QPC = (B * T) // NCORES          # 512 query rows per core
P = 128                          # partitions
TQ_TILES = QPC // P              # 4 tiles of 128 query rows
S_CHUNKS = T // P                # 16 contraction chunks
EPS_SUM = np.float32(1e-8) * np.float32(16.0)  # EPS in head-sum domain

_compiled = {}


def _build_nc():
    import concourse.bass as bass
    import concourse.bacc as bacc
    import concourse.mybir as mybir
    import concourse.tile as tile
    from concourse import masks

    f32 = mybir.dt.float32
    bf16 = mybir.dt.bfloat16
    nc = bacc.Bacc(
        "TRN2",
        target_bir_lowering=False,
        debug=False,
        enable_asserts=False,
        num_devices=NCORES,
    )
    attn = nc.dram_tensor("attn", [Hh, QPC, T], f32, kind="ExternalInput").ap()
    mlp = nc.dram_tensor("mlp", [T, H], f32, kind="ExternalInput").ap()
    out = nc.dram_tensor("out", [QPC, H], f32, kind="ExternalOutput").ap()

    with tile.TileContext(nc) as tc:
        with (
            tc.tile_pool(name="persist", bufs=1) as persist,
            tc.tile_pool(name="heads", bufs=13) as heads,
            tc.tile_pool(name="acc", bufs=3) as accp,
            tc.tile_pool(name="stage", bufs=4) as stagep,
            tc.tile_pool(name="wm", bufs=1) as wmp,
            tc.tile_pool(name="wmt", bufs=1) as wmtp,
            tc.tile_pool(name="small", bufs=2) as small,
            tc.tile_pool(name="outsb", bufs=2) as outsbp,
            tc.tile_pool(name="tp_psum", bufs=4, space="PSUM") as tp_psum,
            tc.tile_pool(name="mm_psum", bufs=2, space="PSUM") as mm_psum,
        ):
            mlp_sb = persist.tile([P, S_CHUNKS, H], bf16)
            ident = persist.tile([P, P], f32)
            masks.make_identity(nc, ident[:])

            mlp_next = [0]

            def load_mlp_group(n):
                # fp32 chunks staged via HWDGE, cast bf16 on ScalarE
                for _ in range(n):
                    c = mlp_next[0]
                    if c >= S_CHUNKS:
                        return
                    mlp_next[0] += 1
                    st = stagep.tile([P, H], f32, tag="st")
                    nc.sync.dma_start(out=st, in_=mlp[c * P : (c + 1) * P, :])
                    nc.scalar.copy(mlp_sb[:, c, :], st)

            def transpose_half(wm, wmt, half):
                for g in range(2 * half, 2 * half + 2):
                    pt = tp_psum.tile([P, 4 * P], f32, tag="pt")
                    for j in range(4):
                        c = 4 * g + j
                        nc.tensor.transpose(
                            pt[:, j * P : (j + 1) * P],
                            wm[:, c * P : (c + 1) * P],
                            ident[:],
                        )
                    nc.scalar.copy(wmt[:, 4 * g : 4 * g + 4, :], pt[:])

            def epilogue(acc, mx, q):
                # mask with the row's 8th-largest (two s-halves so the
                # TensorEngine can start transposing early), renormalize,
                # transpose, contract, store.
                HF = T // 2
                v8 = mx[:, K - 1 : K]
                wm = wmp.tile([P, T], f32, tag="wm")
                wmt = wmtp.tile([P, S_CHUNKS, P], bf16, tag="wmt")
                ssum = small.tile([P, 2], f32, tag="ssum")
                for half in range(2):
                    sl = slice(half * HF, (half + 1) * HF)
                    nc.vector.scalar_tensor_tensor(
                        out=wm[:, sl],
                        in0=acc[:, sl],
                        scalar=v8,
                        in1=acc[:, sl],
                        op0=mybir.AluOpType.is_ge,
                        op1=mybir.AluOpType.mult,
                        accum_out=ssum[:, half : half + 1],
                    )
                    transpose_half(wm, wmt, half)
                stot = small.tile([P, 1], f32, tag="stot")
                nc.vector.tensor_add(out=stot, in0=ssum[:, 0:1], in1=ssum[:, 1:2])
                nc.vector.tensor_scalar_max(stot, stot, float(EPS_SUM))
                rcp = small.tile([P, 1], f32, tag="rcp")
                nc.vector.reciprocal(rcp, stot)

                acc_ps = mm_psum.tile([P, H], f32, tag="acc_ps")
                osb = outsbp.tile([P, H], f32, tag="osb")
                for c in range(S_CHUNKS):
                    for nh in range(H // 512):
                        nsl = slice(nh * 512, (nh + 1) * 512)
                        nc.tensor.matmul(
                            acc_ps[:, nsl],
                            lhsT=wmt[:, c, :],
                            rhs=mlp_sb[:, c, nsl],
                            start=(c == 0),
                            stop=(c == S_CHUNKS - 1),
                        )
                for nh in range(H // 512):
                    nsl = slice(nh * 512, (nh + 1) * 512)
                    nc.scalar.activation(
                        out=osb[:, nsl],
                        in_=acc_ps[:, nsl],
                        func=mybir.ActivationFunctionType.Copy,
                        scale=rcp[:, :],
                    )
                    nc.scalar.dma_start(out=out[q, nsl], in_=osb[:, nsl])

            pending = []  # deferred (acc, mx, q) epilogues

            def run_pending():
                if pending:
                    epilogue(*pending.pop())

            def accumulate(acc, q, sl, mlp_every=0):
                nc.sync.dma_start(out=acc[:, sl], in_=attn[0, q, sl])
                for h in range(1, Hh):
                    ht = heads.tile([P, sl.stop - sl.start], f32, tag="ht")
                    nc.sync.dma_start(out=ht, in_=attn[h, q, sl])
                    nc.vector.tensor_add(
                        out=acc[:, sl], in0=acc[:, sl], in1=ht
                    )
                    if mlp_every and h % mlp_every == 0:
                        load_mlp_group(4)

            for t in range(TQ_TILES - 1):
                q = slice(t * P, (t + 1) * P)
                acc = accp.tile([P, T], f32, tag="acc")
                # interleave the 16 mlp chunk loads into tiles 0-1 in groups
                # of 4 (= stage bufs), finishing well before tile 0's matmuls
                accumulate(acc, q, slice(0, T), mlp_every=5 if t < 2 else 0)
                mx = small.tile([P, K], f32, tag="mx")
                nc.vector.max(out=mx, in_=acc)
                run_pending()
                pending.append((acc, mx, q))

            # last tile: stream s-halves; the first half's top-8 scan and the
            # second-to-last tile's epilogue hide under the second half's
            # streaming, so only this tile's epilogue trails the stream
            t = TQ_TILES - 1
            q = slice(t * P, (t + 1) * P)
            HF = T // 2
            acc = accp.tile([P, T], f32, tag="acc")
            accumulate(acc, q, slice(0, HF))
            mx1 = small.tile([P, K], f32, tag="mx")
            nc.vector.max(out=mx1, in_=acc[:, :HF])
            run_pending()
            accumulate(acc, q, slice(HF, T))
            mx2 = small.tile([P, K], f32, tag="mx2")
            nc.vector.max(out=mx2, in_=acc[:, HF:])
            mg = small.tile([P, 2 * K], f32, tag="mg")
            nc.vector.tensor_copy(mg[:, :K], mx1)
            nc.vector.tensor_copy(mg[:, K:], mx2)
            g8 = small.tile([P, K], f32, tag="g8")
            nc.vector.max(out=g8, in_=mg)
            epilogue(acc, g8, q)

    nc.compile()
    return nc


def _get_nc():
    if "nc" not in _compiled:
        _compiled["nc"] = _build_nc()
    return _compiled["nc"]


def kernel(mlp_hidden: np.ndarray, attn_weights: np.ndarray) -> np.ndarray:
    from concourse.bass_utils import run_bass_kernel_spmd

    mlp_hidden = np.ascontiguousarray(mlp_hidden, dtype=np.float32)
    attn_weights = np.ascontiguousarray(attn_weights, dtype=np.float32)
    assert mlp_hidden.shape == (B, T, H)
    assert attn_weights.shape == (B, Hh, T, T)

    nc = _get_nc()
    in_maps = []
    for c in range(NCORES):
        b = c // (NCORES // B)
        q0 = (c % (NCORES // B)) * QPC
        in_maps.append(
            {
                "attn": np.ascontiguousarray(attn_weights[b, :, q0 : q0 + QPC, :]),
                "mlp": mlp_hidden[b],
            }
        )
    res = run_bass_kernel_spmd(nc, in_maps, list(range(NCORES)))
    out = np.empty((B, T, H), dtype=np.float32)
    for c in range(NCORES):
        b = c // (NCORES // B)
        q0 = (c % (NCORES // B)) * QPC
        out[b, q0 : q0 + QPC] = res.results[c]["out"]
    return out
